# revision 1
# baseline (speedup 1.0000x reference)
"""GQA attention block (QKV proj + causal attention + output proj) on 8 trn2 cores.

Sharding: core c -> (batch b = c//4, kv-group g = c%4). Each core computes 4 Q
heads (one KV-head group) of one batch and a partial o_proj output; the host
sums the 4 partials per batch (row-sharded o_proj all-reduce done host-side).

Matmul inputs are bf16 (1 cycle/row on the PE vs 4 for fp32); accumulation is
fp32 in PSUM. All device inputs are pre-tiled host-side to [128, ko, ...] so
every DMA is a full-bandwidth contiguous-per-partition transfer. Phase 1 runs
tcol-major with six concurrent full-depth PSUM accumulation groups, paced by
the streaming x^T DMA. Attention uses transposed scores S^T[tk, tq] so the
softmax denominator comes for free from a ones-column appended to V;
scores/exp/AV are software-pipelined (scores three tk-blocks ahead) so the PE
never waits on the Scalar engine's exp. The o_proj partial for each 512-query
chunk is emitted one chunk late so its matmuls never wait on fresh YT writes.
"""

import math

import numpy as np

# Model dims (hardcoded per contract; kernel.py must be self-contained).
B = 2
T = 2048
E = 2048
HD = 128               # head dim
NH = 16                # query heads total
NKV = 4                # kv heads total
NHC = 4                # query heads per core
P = 128
KO = E // P            # 16 contraction subtiles of 128
TQC = T // 512         # 4 query chunks of 512
TB = T // P            # 16 t blocks of 128
SCALE = 1.0 / math.sqrt(HD)
N_CORES = 8

_NC_CACHE = {}


def _build_nc(loop_n=1, acc=2, ybufs=5, tbufs=1, ahead=3):
    import concourse.bacc as bacc
    import concourse.mybir as mybir
    import concourse.tile as tile
    from concourse.masks import make_identity, make_upper_triangular

    f32 = mybir.dt.float32
    bf16 = mybir.dt.bfloat16
    nc = bacc.Bacc(None, target_bir_lowering=False)

    # Inputs are host-pre-tiled: [128 partitions, ko, chunk] with the e
    # (contraction) axis split as e = ko*128 + p.
    xT3 = nc.dram_tensor("xT3", [P, KO, T], bf16, kind="ExternalInput")
    wqT3 = nc.dram_tensor("wqT3", [P, KO, NHC * HD], bf16, kind="ExternalInput")
    wkT3 = nc.dram_tensor("wkT3", [P, KO, HD], bf16, kind="ExternalInput")
    wvT3 = nc.dram_tensor("wvT3", [P, KO, HD], bf16, kind="ExternalInput")
    woT3 = nc.dram_tensor("woT3", [P, NHC, E], bf16, kind="ExternalInput")
    out = nc.dram_tensor("out", [T, E], bf16, kind="ExternalOutput")

    out_r = out.rearrange("(tb p) e -> p tb e", p=P)      # [128, 16, E]

    with tile.TileContext(nc) as tc:
        if loop_n > 1:
            # Bench-only: run the whole (idempotent) kernel body loop_n
            # times device-side so one NEFF execution measures steady-state
            # per-iteration device time.
            with tc.For_i(0, loop_n):
                _emit_body(nc, tc, mybir, tile, make_identity,
                           make_upper_triangular, f32, bf16,
                           xT3, wqT3, wkT3, wvT3, woT3, out_r,
                           acc, ybufs, tbufs, ahead)
        else:
            _emit_body(nc, tc, mybir, tile, make_identity,
                       make_upper_triangular, f32, bf16,
                       xT3, wqT3, wkT3, wvT3, woT3, out_r,
                       acc, ybufs, tbufs, ahead)

    nc.finalize()
    return nc


def _emit_body(nc, tc, mybir, tile, make_identity, make_upper_triangular,
               f32, bf16, xT3, wqT3, wkT3, wvT3, woT3, out_r,
               acc=2, ybufs=5, tbufs=1, ahead=3):
    if True:
        with (
            tc.tile_pool(name="const", bufs=1) as constp,
            tc.tile_pool(name="qkv", bufs=1) as qkvp,
            tc.tile_pool(name="ps_t", bufs=tbufs, space="PSUM") as ps_t,
        ):
            identity = constp.tile([P, P], bf16, tag="ident")
            make_identity(nc, identity)

            # tri[p, q] = 1.0 where p <= q — causal mask for the one
            # tk==tq diagonal 128x128 sub-block.
            tri = constp.tile([P, P], bf16, tag="tri")
            make_upper_triangular(nc, tri[:], val=1.0, diag=True)

            QT = qkvp.tile([P, NHC, T], bf16, tag="QT")    # q^T per head [d, t]
            KT = qkvp.tile([P, T], bf16, tag="KT")         # k^T [d, t]
            VT = qkvp.tile([P, T], bf16, tag="VT")         # v^T [d, t]
            VAUG = qkvp.tile([P, TB, HD + 1], bf16, tag="VAUG")  # v blocks [tk, 129]
            YT = qkvp.tile([P, NHC, T], bf16, tag="YT")    # y^T per head [d, t]
            WOT = qkvp.tile([P, NHC, E], bf16, tag="WOT")

            nc.vector.memset(VAUG[:, :, HD:HD + 1], 1.0)

            def make_vaug(tcol):
                # v^T -> v natural layout blocks (ones column for the
                # softmax denominator comes from the memset above).
                for tb in range(4 * tcol, 4 * tcol + 4):
                    pst = ps_t.tile([P, P], bf16, tag="ps_t")
                    nc.tensor.transpose(
                        pst[:], VT[:, tb * P:(tb + 1) * P], identity[:]
                    )
                    nc.vector.tensor_copy(VAUG[:, tb, 0:HD], pst[:])

            # ---- Phase 1: projections. q^T/k^T/v^T = W @ x^T, contracting
            # over e with full-depth (K=2048) PSUM accumulation, six output
            # chunks (K, V, Q0..Q3 for one tcol) in flight at once and the
            # e-subtile loop innermost so compute tracks the x^T DMA stream.
            with (
                tc.tile_pool(name="w1", bufs=1) as w1p,
                tc.tile_pool(name="ps_proj", bufs=6, space="PSUM") as ps_proj,
            ):
                XT = w1p.tile([P, KO, T], bf16, tag="XT")
                WQT = w1p.tile([P, KO, NHC * HD], bf16, tag="WQT")
                WKT = w1p.tile([P, KO, HD], bf16, tag="WKT")
                WVT = w1p.tile([P, KO, HD], bf16, tag="WVT")

                # DMA order sets the critical path: K/V weights and the first
                # Q-weight chunk, then x^T streamed per (ko, half-T) so the
                # first matmul starts ~5us in and compute tracks the stream.
                nc.sync.dma_start(WKT[:], wkT3[:])
                nc.sync.dma_start(WVT[:], wvT3[:])
                nc.sync.dma_start(WQT[:, 0:4], wqT3[:, 0:4])
                for ko in range(KO):
                    nc.sync.dma_start(XT[:, ko, 0:1024], xT3[:, ko, 0:1024])
                    if ko % 4 == 3 and ko < 12:
                        q = ko // 4 + 1
                        nc.sync.dma_start(
                            WQT[:, 4 * q:4 * (q + 1)], wqT3[:, 4 * q:4 * (q + 1)]
                        )
                for ko in range(KO):
                    nc.sync.dma_start(XT[:, ko, 1024:2048], xT3[:, ko, 1024:2048])
                for h in range(NHC):
                    nc.sync.dma_start(WOT[:, h], woT3[:, h])

                # Touch the Exp table now so the one-time activation-table
                # load happens during the x^T DMA stream, not at the first
                # real exp in phase 2.
                warm = w1p.tile([P, 1], f32, tag="warm")
                nc.scalar.activation(
                    warm[:], WKT[:, 0, 0:1],
                    mybir.ActivationFunctionType.Exp, scale=0.0,
                )

                # Keep the PE busy while x^T streams in: back-to-back dummy
                # transposes release the HAM clock throttle so the real
                # matmuls start at full clock instead of ramping.
                for _ in range(40):
                    pst = ps_t.tile([P, P], bf16, tag="ps_t")
                    nc.tensor.transpose(pst[:], identity[:], identity[:])

                for tcol in range(TQC):
                    cols = slice(tcol * 512, (tcol + 1) * 512)
                    psK = ps_proj.tile([P, 512], f32, tag="ps_proj", name="psK")
                    psV = ps_proj.tile([P, 512], f32, tag="ps_proj", name="psV")
                    psQ = [
                        ps_proj.tile([P, 512], f32, tag="ps_proj", name=f"psQ{h}")
                        for h in range(NHC)
                    ]
                    for ko in range(KO):
                        st = ko == 0
                        sp = ko == KO - 1
                        xk = XT[:, ko, cols]
                        nc.tensor.matmul(psK[:], WKT[:, ko], xk, start=st, stop=sp)
                        nc.tensor.matmul(psV[:], WVT[:, ko], xk, start=st, stop=sp)
                        for h in range(NHC):
                            nc.tensor.matmul(
                                psQ[h][:], WQT[:, ko, h * HD:(h + 1) * HD], xk,
                                start=st, stop=sp,
                            )
                    nc.vector.tensor_copy(KT[:, cols], psK[:])
                    nc.vector.tensor_copy(VT[:, cols], psV[:])
                    for h in range(NHC):
                        nc.vector.tensor_copy(QT[:, h, cols], psQ[h][:])
                    if tcol < TQC - 1:
                        make_vaug(tcol)

            # ---- Phases 2+3: causal attention (transposed scores), with the
            # o_proj partial for query chunk tqc-1 emitted after chunk tqc's
            # attention so its matmuls never wait on fresh YT.
            with (
                tc.tile_pool(name="work", bufs=6) as work,
                tc.tile_pool(name="nwork", bufs=4) as nwork,
                tc.tile_pool(name="owork", bufs=4) as owork,
                tc.tile_pool(name="ps_acc", bufs=acc, space="PSUM") as ps_acc,
                tc.tile_pool(name="ps_y", bufs=ybufs, space="PSUM") as ps_y,
            ):
                def oproj_chunk(tqc):
                    # out[t, e] = sum_h y_h^T.T @ woT_h for 4 t-blocks
                    for tb in range(4 * tqc, 4 * tqc + 4):
                        for ec in range(4):
                            ps = ps_acc.tile([P, 512], f32, tag="ps_acc")
                            for h2 in range(NHC):
                                nc.tensor.matmul(
                                    ps[:],
                                    YT[:, h2, tb * P:(tb + 1) * P],
                                    WOT[:, h2, ec * 512:(ec + 1) * 512],
                                    start=(h2 == 0),
                                    stop=(h2 == NHC - 1),
                                )
                            osb = owork.tile([P, 512], bf16, tag="osb")
                            nc.vector.tensor_copy(osb[:], ps[:])
                            nc.sync.dma_start(
                                out_r[:, tb, ec * 512:(ec + 1) * 512], osb[:]
                            )

                # Last V transposes here: their PE work covers the PSUM
                # pool handoff (first scores wait on phase 1's last copies).
                make_vaug(TQC - 1)

                for tqc in range(TQC):
                    ntk = 4 * (tqc + 1)   # tk blocks up to the diagonal
                    for h in range(NHC):

                        def scores_exp(tk):
                            # S^T[tk, tq] for the causally-valid tq columns,
                            # exp'd into bf16; the single diagonal 128x128
                            # sub-block gets the triangular mask.
                            i = tk - 4 * tqc
                            off = max(0, i) * P
                            w = 512 - off
                            pss = ps_acc.tile([P, 512], f32, tag="ps_acc")
                            nc.tensor.matmul(
                                pss[:, 0:w],
                                KT[:, tk * P:(tk + 1) * P],
                                QT[:, h, tqc * 512 + off:(tqc + 1) * 512],
                                start=True,
                                stop=True,
                            )
                            es = work.tile([P, 512], bf16, tag="expS")
                            nc.scalar.activation(
                                es[:, 0:w], pss[:, 0:w],
                                mybir.ActivationFunctionType.Exp,
                                scale=SCALE,
                            )
                            if i >= 0:
                                nc.vector.tensor_mul(
                                    out=es[:, 0:P], in0=es[:, 0:P], in1=tri[:]
                                )
                            return es

                        # One psy accumulator per PSUM bank: a matmul with
                        # start=True clears has_written for its whole bank,
                        # so accumulation groups must never share one.
                        psy = [
                            ps_y.tile([P, HD + 1], f32, tag="ps_y",
                                      name=f"psy_{j}")[:]
                            for j in range(4)
                        ]
                        # scores/exp run three tk blocks ahead of AV so the
                        # PE never waits on the Scalar engine.
                        pipe = {}
                        for tk in range(min(ahead, ntk)):
                            pipe[tk] = scores_exp(tk)
                        ysbs = [None] * 4
                        for tk in range(ntk):
                            if tk + ahead < ntk:
                                pipe[tk + ahead] = scores_exp(tk + ahead)
                            i = tk - 4 * tqc
                            off = max(0, i) * P
                            es = pipe.pop(tk)
                            for j in range(max(0, i), 4):
                                nc.tensor.matmul(
                                    psy[j],
                                    es[:, j * P - off:(j + 1) * P - off],
                                    VAUG[:, tk],
                                    start=(tk == 0),
                                    stop=(tk == 4 * tqc + j),
                                )
                            if i >= 0:
                                # psy[i] just took its last accumulation:
                                # normalize now (DVE overlaps the remaining
                                # AV matmuls) so its bank frees early for
                                # the next head's accumulation.
                                recip = nwork.tile([P, 1], f32, tag="recip")
                                nc.vector.reciprocal(
                                    recip[:], psy[i][:, HD:HD + 1]
                                )
                                ysb = nwork.tile([P, P], bf16, tag="ysb")
                                nc.vector.tensor_scalar_mul(
                                    ysb[:], psy[i][:, 0:HD], recip[:]
                                )
                                ysbs[i] = ysb
                        for j in range(4):
                            jg = 4 * tqc + j
                            pst = ps_t.tile([P, P], bf16, tag="ps_t")
                            nc.tensor.transpose(pst[:], ysbs[j][:], identity[:])
                            nc.vector.tensor_copy(
                                YT[:, h, jg * P:(jg + 1) * P], pst[:]
                            )

                    if tqc > 0:
                        oproj_chunk(tqc - 1)
                oproj_chunk(TQC - 1)


def _get_nc():
    if "nc" not in _NC_CACHE:
        _NC_CACHE["nc"] = _build_nc()
    return _NC_CACHE["nc"]


def _tile_e(arr, chunk):
    # [out_dim, E] -> transpose -> [E, out_dim] -> [128, KO, out_dim]
    import ml_dtypes

    t = np.ascontiguousarray(arr.T)
    t = t.reshape(KO, P, chunk).transpose(1, 0, 2)
    return np.ascontiguousarray(t).astype(ml_dtypes.bfloat16)


def _in_maps(x, wq, wk, wv, wo):
    import ml_dtypes

    bf16 = ml_dtypes.bfloat16
    xT3 = [_tile_e(x[b], T) for b in range(B)]
    wqT3 = [_tile_e(wq[g * 512:(g + 1) * 512], 512) for g in range(NKV)]
    wkT3 = [_tile_e(wk[g * HD:(g + 1) * HD], HD) for g in range(NKV)]
    wvT3 = [_tile_e(wv[g * HD:(g + 1) * HD], HD) for g in range(NKV)]
    # wo columns for group g, transposed to [512, E] then tiled to [128,4,E]
    woT3 = []
    for g in range(NKV):
        t = np.ascontiguousarray(wo[:, g * 512:(g + 1) * 512].T)  # [512, E]
        t = t.reshape(NHC, P, E).transpose(1, 0, 2)
        woT3.append(np.ascontiguousarray(t).astype(bf16))
    maps = []
    for c in range(N_CORES):
        b, g = divmod(c, NKV)
        maps.append({
            "xT3": xT3[b],
            "wqT3": wqT3[g],
            "wkT3": wkT3[g],
            "wvT3": wvT3[g],
            "woT3": woT3[g],
        })
    return maps


def kernel(x, wq, wk, wv, wo):
    from concourse.bass_utils import run_bass_kernel_spmd

    x = np.asarray(x, dtype=np.float32)
    wq = np.asarray(wq, dtype=np.float32)
    wk = np.asarray(wk, dtype=np.float32)
    wv = np.asarray(wv, dtype=np.float32)
    wo = np.asarray(wo, dtype=np.float32)

    nc = _get_nc()
    in_maps = _in_maps(x, wq, wk, wv, wo)

    res = run_bass_kernel_spmd(nc, in_maps, core_ids=list(range(N_CORES)))

    partials = [np.asarray(res.results[c]["out"]).astype(np.float32)
                for c in range(N_CORES)]
    out = np.empty((B, T, E), dtype=np.float32)
    for b in range(B):
        acc = partials[NKV * b]
        for g in range(1, NKV):
            acc = acc + partials[NKV * b + g]
        out[b] = acc
    return out

